# revision 1
# baseline (speedup 1.0000x reference)
"""GroupedQueryAttention on 8 Trainium2 NeuronCores.

Sharding: core c = 4*b + g handles batch b (of 2) and KV group g (of 4),
i.e. 4 query heads (512 q-dims) + one 128-dim K/V head. o_proj is computed
as per-group partials (transposed layout) and summed with a ReduceScatter
across the 4 cores of each batch; each core ends up with a 512-row chunk
of the transposed output, which the host reassembles.

All matmuls run in fp16 (1 PE cycle/row) with fp32 PSUM accumulation.
Layouts are chosen so softmax is computed over the PSUM free dim never
needing a transpose of the big P matrix:
  - projections produce Q^T/K^T directly (lhsT=W tile, rhs=x^T tile)
  - scores are computed as S^T = (K^T).T @ Q^T
  - exp(S^T) = P^T feeds A@V as lhsT directly
  - V carries an extra ones-column so the softmax denominator falls out
    of the A@V matmul for free; normalization is applied to the small
    A@V output rather than to P.
"""

import math
import sys

import numpy as np

sys.path.insert(0, "/opt/trn_rl_repo")

B = 2
T = 2048
D = 2048
HEADS = 16
GROUPS = 4
HD = 128  # head dim
M = HEADS // GROUPS  # heads per group = 4
GQ = M * HD  # q dims per group = 512
SCALE = 1.0 / math.sqrt(HD)
N_CORES = 8
TCH = 512  # t chunk
NTCH = T // TCH  # 4
NSB = T // 128  # 16 s blocks
NKS = D // 128  # 16 contraction steps for projections

_COMPILED = {}


def _build():
    import concourse.bass as bass
    import concourse.mybir as mybir
    import concourse.tile as tile
    from concourse import bacc
    from concourse.masks import make_identity

    f16 = mybir.dt.float16
    f32 = mybir.dt.float32
    Exp = mybir.ActivationFunctionType.Exp
    Identity = mybir.ActivationFunctionType.Identity
    Copy = mybir.ActivationFunctionType.Copy

    nc = bacc.Bacc("TRN2", target_bir_lowering=False, num_devices=N_CORES)

    xT = nc.declare_dram_parameter("xT", [D, T], f16, isOutput=False)
    wq = nc.declare_dram_parameter("wq", [D, GQ], f16, isOutput=False)
    wk = nc.declare_dram_parameter("wk", [D, HD], f16, isOutput=False)
    wv = nc.declare_dram_parameter("wv", [D, HD], f16, isOutput=False)
    wo = nc.declare_dram_parameter("wo", [GQ, D], f16, isOutput=False)
    bqs_d = nc.declare_dram_parameter("bqs", [128, M], f32, isOutput=False)
    bks_d = nc.declare_dram_parameter("bks", [128, 1], f32, isOutput=False)
    bvs_d = nc.declare_dram_parameter("bvs", [128, 1], f32, isOutput=False)
    bo4_d = nc.declare_dram_parameter("bo4", [128, D // 128], f32, isOutput=False)
    outT = nc.declare_dram_parameter("outT", [D, TCH], f32, isOutput=True)

    groups = [[0, 1, 2, 3], [4, 5, 6, 7]]

    with tile.TileContext(nc) as tc:
        with (
            tc.tile_pool(name="const", bufs=1) as const,
            tc.tile_pool(name="work", bufs=2) as work,
            tc.tile_pool(name="psum", bufs=1, space="PSUM") as psum,
            tc.tile_pool(name="dram", bufs=1, space="DRAM") as dram,
        ):
            ident = const.tile([128, 128], f16)
            make_identity(nc, ident)
            bqs = const.tile([128, M], f32)
            bks = const.tile([128, 1], f32)
            bvs = const.tile([128, 1], f32)
            bo4 = const.tile([128, D // 128], f32)
            nc.sync.dma_start(bqs[:], bqs_d[:])
            nc.sync.dma_start(bks[:], bks_d[:])
            nc.sync.dma_start(bvs[:], bvs_d[:])
            nc.sync.dma_start(bo4[:], bo4_d[:])

            xt = const.tile([128, NKS, T], f16)
            wq_sb = const.tile([128, NKS, GQ], f16)
            wk_sb = const.tile([128, NKS, HD], f16)
            wv_sb = const.tile([128, NKS, HD], f16)
            wo_sb = const.tile([128, M, D], f16)
            for i in range(NKS):
                nc.sync.dma_start(xt[:, i, :], xT[i * 128 : (i + 1) * 128, :])
                nc.sync.dma_start(wq_sb[:, i, :], wq[i * 128 : (i + 1) * 128, :])
                nc.sync.dma_start(wk_sb[:, i, :], wk[i * 128 : (i + 1) * 128, :])
                nc.sync.dma_start(wv_sb[:, i, :], wv[i * 128 : (i + 1) * 128, :])
            for h in range(M):
                nc.sync.dma_start(wo_sb[:, h, :], wo[h * 128 : (h + 1) * 128, :])

            qt = const.tile([128, M, T], f16)
            kt = const.tile([128, T], f16)
            vt_sb = const.tile([128, T], f16)
            v_sb = const.tile([128, NSB, 132], f16)

            # ---- projections ----
            for h in range(M):
                for tc_i in range(NTCH):
                    acc = psum.tile([128, TCH], f32, tag="acc", bufs=3, name="acc")
                    for ks in range(NKS):
                        nc.tensor.matmul(
                            acc[:],
                            wq_sb[:, ks, h * 128 : (h + 1) * 128],
                            xt[:, ks, tc_i * TCH : (tc_i + 1) * TCH],
                            start=(ks == 0),
                            stop=(ks == NKS - 1),
                        )
                    nc.vector.tensor_scalar(
                        qt[:, h, tc_i * TCH : (tc_i + 1) * TCH],
                        acc[:],
                        SCALE,
                        bqs[:, h : h + 1],
                        op0=mybir.AluOpType.mult,
                        op1=mybir.AluOpType.add,
                    )
            for tc_i in range(NTCH):
                acc = psum.tile([128, TCH], f32, tag="acc", bufs=3, name="acc")
                for ks in range(NKS):
                    nc.tensor.matmul(
                        acc[:],
                        wk_sb[:, ks, :],
                        xt[:, ks, tc_i * TCH : (tc_i + 1) * TCH],
                        start=(ks == 0),
                        stop=(ks == NKS - 1),
                    )
                nc.vector.tensor_scalar_add(
                    kt[:, tc_i * TCH : (tc_i + 1) * TCH], acc[:], bks[:, 0:1]
                )
            for tc_i in range(NTCH):
                acc = psum.tile([128, TCH], f32, tag="acc", bufs=3, name="acc")
                for ks in range(NKS):
                    nc.tensor.matmul(
                        acc[:],
                        wv_sb[:, ks, :],
                        xt[:, ks, tc_i * TCH : (tc_i + 1) * TCH],
                        start=(ks == 0),
                        stop=(ks == NKS - 1),
                    )
                nc.vector.tensor_scalar_add(
                    vt_sb[:, tc_i * TCH : (tc_i + 1) * TCH], acc[:], bvs[:, 0:1]
                )
            # V natural [s, hd] + ones column for the denominator
            for s in range(NSB):
                tp = psum.tile([128, 128], f16, tag="tp", bufs=1, name="tp")
                nc.tensor.transpose(tp[:], vt_sb[:, s * 128 : (s + 1) * 128], ident[:])
                nc.vector.tensor_copy(v_sb[:, s, 0:128], tp[:])
            nc.vector.memset(v_sb[:, :, 128:129], 1.0)

            # ---- attention + o_proj, streamed per t-chunk ----
            partial = dram.tile([NTCH * D, TCH], f32, tag="ptl", name="partial")
            for tc_i in range(NTCH):
                at = work.tile([128, M, TCH], f16, tag="at", bufs=2, name="at")
                for h in range(M):
                    opks = [
                        psum.tile([128, 129], f32, tag="opk", bufs=4, name=f"opk{i}")
                        for i in range(4)
                    ]
                    for s in range(NSB):
                        sps = psum.tile([128, TCH], f32, tag="acc", bufs=3, name="sps")
                        nc.tensor.matmul(
                            sps[:],
                            kt[:, s * 128 : (s + 1) * 128],
                            qt[:, h, tc_i * TCH : (tc_i + 1) * TCH],
                            start=True,
                            stop=True,
                        )
                        p_sb = work.tile([128, TCH], f16, tag="p", bufs=3, name="p_sb")
                        nc.scalar.activation(p_sb[:], sps[:], Exp)
                        for tb in range(4):
                            nc.tensor.matmul(
                                opks[tb][:, 0:129],
                                p_sb[:, tb * 128 : (tb + 1) * 128],
                                v_sb[:, s, 0:129],
                                start=(s == 0),
                                stop=(s == NSB - 1),
                            )
                    for tb in range(4):
                        opk = opks[tb]
                        off = 0
                        rcp = work.tile([128, 1], f32, tag="rcp", bufs=2, name="rcp")
                        nc.vector.reciprocal(rcp[:], opk[:, off + 128 : off + 129])
                        o_sb = work.tile([128, 128], f16, tag="osb", bufs=2, name="osb")
                        nc.vector.tensor_scalar_mul(
                            o_sb[:], opk[:, off : off + 128], rcp[:]
                        )
                        tp = psum.tile([128, 128], f16, tag="tp", bufs=1, name="tp")
                        nc.tensor.transpose(tp[:], o_sb[:], ident[:])
                        nc.vector.tensor_copy(
                            at[:, h, tb * 128 : (tb + 1) * 128], tp[:]
                        )
                # o_proj partial (transposed): partial^T[c, t] for this t-chunk
                for cb in range(D // 128):
                    pp = psum.tile([128, TCH], f32, tag="acc", bufs=3, name="pp")
                    for h in range(M):
                        nc.tensor.matmul(
                            pp[:],
                            wo_sb[:, h, cb * 128 : (cb + 1) * 128],
                            at[:, h, :],
                            start=(h == 0),
                            stop=(h == M - 1),
                        )
                    po_sb = work.tile([128, TCH], f32, tag="po", bufs=3, name="po_sb")
                    nc.vector.tensor_scalar_add(po_sb[:], pp[:], bo4[:, cb : cb + 1])
                    nc.sync.dma_start(
                        partial[
                            tc_i * D + cb * 128 : tc_i * D + (cb + 1) * 128, :
                        ],
                        po_sb[:],
                    )
            rs = dram.tile([D, TCH], f32, tag="rs", name="rs")
            nc.gpsimd.collective_compute(
                "ReduceScatter",
                mybir.AluOpType.add,
                replica_groups=groups,
                ins=[partial[:]],
                outs=[rs[:]],
            )
            nc.sync.dma_start(outT[:], rs[:])

    nc.compile()
    return nc


def _get_nc():
    if "nc" not in _COMPILED:
        _COMPILED["nc"] = _build()
    return _COMPILED["nc"]


def kernel(x, Wq, bq, Wk, bk, Wv, bv, Wo, bo):
    from concourse.bass_utils import run_bass_kernel_spmd

    x = np.asarray(x, np.float32)
    Wq = np.asarray(Wq, np.float32)
    Wk = np.asarray(Wk, np.float32)
    Wv = np.asarray(Wv, np.float32)
    Wo = np.asarray(Wo, np.float32)
    bq = np.asarray(bq, np.float32)
    bk = np.asarray(bk, np.float32)
    bv = np.asarray(bv, np.float32)
    bo = np.asarray(bo, np.float32)

    nc = _get_nc()

    in_maps = []
    for c in range(N_CORES):
        b, g = c // 4, c % 4
        in_maps.append(
            {
                "xT": np.ascontiguousarray(x[b].T).astype(np.float16),
                "wq": np.ascontiguousarray(
                    Wq[:, g * GQ : (g + 1) * GQ]
                ).astype(np.float16),
                "wk": np.ascontiguousarray(
                    Wk[:, g * HD : (g + 1) * HD]
                ).astype(np.float16),
                "wv": np.ascontiguousarray(
                    Wv[:, g * HD : (g + 1) * HD]
                ).astype(np.float16),
                "wo": np.ascontiguousarray(
                    Wo[g * GQ : (g + 1) * GQ, :]
                ).astype(np.float16),
                "bqs": np.ascontiguousarray(
                    (bq[g * GQ : (g + 1) * GQ] * SCALE).reshape(M, 128).T
                ),
                "bks": np.ascontiguousarray(
                    bk[g * HD : (g + 1) * HD].reshape(1, 128).T
                ),
                "bvs": np.ascontiguousarray(
                    bv[g * HD : (g + 1) * HD].reshape(1, 128).T
                ),
                "bo4": np.ascontiguousarray((bo / 4.0).reshape(D // 128, 128).T),
            }
        )

    res = run_bass_kernel_spmd(nc, in_maps, list(range(N_CORES)))
    _COMPILED["last_res"] = res

    out = np.empty((B, T, D), np.float32)
    for b in range(B):
        for r in range(4):
            out[b, r * TCH : (r + 1) * TCH, :] = res.results[4 * b + r]["outT"].T
    return out



# revision 15
# speedup vs baseline: 1.4200x; 1.4200x over previous
"""GroupedQueryAttention on 8 Trainium2 NeuronCores.

Sharding: core c = 4*b + g handles batch b (of 2) and KV group g (of 4),
i.e. 4 query heads (512 q-dims) + one 128-dim K/V head. o_proj is computed
as per-group partials (transposed layout) and summed with per-t-chunk
ReduceScatters (fp16) across the 4 cores of each batch, pipelined against
compute of the next t-chunk; each core ends up with a 512-row band of the
transposed output over all T, which the host reassembles.

All matmuls run in fp16 (1 PE cycle/row) with fp32 PSUM accumulation.
Layouts are chosen so softmax is computed over the PSUM free dim never
needing a transpose of the big P matrix:
  - projections produce Q^T/K^T directly (lhsT=W tile, rhs=x^T tile)
  - scores are computed as S^T = (K^T).T @ Q^T
  - exp(S^T) = P^T feeds A@V as lhsT directly
  - V carries an extra ones-column so the softmax denominator falls out
    of the A@V matmul for free; normalization is applied to the small
    A@V output rather than to P.
The attention inner loop is software-pipelined: score matmuls run two
s-blocks ahead of the A@V matmuls so the PE never waits on the exp.
"""

import math
import sys

import numpy as np

sys.path.insert(0, "/opt/trn_rl_repo")

B = 2
T = 2048
D = 2048
HEADS = 16
GROUPS = 4
HD = 128  # head dim
M = HEADS // GROUPS  # heads per group = 4
GQ = M * HD  # q dims per group = 512
SCALE = 1.0 / math.sqrt(HD)
N_CORES = 8
TCH = 512  # t chunk
NTCH = T // TCH  # 4
NSB = T // 128  # 16 s blocks
NKS = D // 128  # 16 contraction steps for projections

_COMPILED = {}


def _build():
    import concourse.bass as bass
    import concourse.mybir as mybir
    import concourse.tile as tile
    from concourse import bacc
    from concourse.masks import make_identity

    f16 = mybir.dt.float16
    f32 = mybir.dt.float32
    Exp = mybir.ActivationFunctionType.Exp

    nc = bacc.Bacc("TRN2", target_bir_lowering=False, num_devices=N_CORES)

    xT = nc.declare_dram_parameter("xT", [D, T], f16, isOutput=False)
    wq = nc.declare_dram_parameter("wq", [D, GQ], f16, isOutput=False)
    wk = nc.declare_dram_parameter("wk", [D, HD], f16, isOutput=False)
    wv = nc.declare_dram_parameter("wv", [D, HD], f16, isOutput=False)
    wo = nc.declare_dram_parameter("wo", [GQ, D], f16, isOutput=False)
    bqs_d = nc.declare_dram_parameter("bqs", [128, M], f32, isOutput=False)
    bks_d = nc.declare_dram_parameter("bks", [128, 1], f32, isOutput=False)
    bvs_d = nc.declare_dram_parameter("bvs", [128, 1], f32, isOutput=False)
    bo4_d = nc.declare_dram_parameter("bo4", [128, D // 128], f32, isOutput=False)
    # core (b, j) outputs band j of out^T: rows [j*512, (j+1)*512), one
    # contiguous [512, 512] block per t-chunk, fp16 (written directly by the
    # per-chunk ReduceScatters)
    outT = nc.declare_dram_parameter("outT", [NTCH, TCH, TCH], f16, isOutput=True)

    groups = [[0, 1, 2, 3], [4, 5, 6, 7]]

    with tile.TileContext(nc) as tc:
        with (
            tc.tile_pool(name="const", bufs=1) as const,
            tc.tile_pool(name="work", bufs=2) as work,
            tc.tile_pool(name="psum", bufs=1, space="PSUM") as psum,
            tc.tile_pool(name="dram", bufs=1, space="DRAM") as dram,
        ):
            ident = const.tile([128, 128], f16)
            make_identity(nc, ident)
            bqs = const.tile([128, M], f32)
            bks = const.tile([128, 1], f32)
            bvs = const.tile([128, 1], f32)
            bo4 = const.tile([128, D // 128], f32)
            nc.sync.dma_start(bqs[:], bqs_d[:])
            nc.sync.dma_start(bks[:], bks_d[:])
            nc.sync.dma_start(bvs[:], bvs_d[:])
            nc.sync.dma_start(bo4[:], bo4_d[:])

            xt = const.tile([128, NKS, T], f16)
            wq_sb = const.tile([128, NKS, GQ], f16)
            wk_sb = const.tile([128, NKS, HD], f16)
            wv_sb = const.tile([128, NKS, HD], f16)
            wo_sb = const.tile([128, M, D], f16)
            for i in range(NKS):
                nc.sync.dma_start(wk_sb[:, i, :], wk[i * 128 : (i + 1) * 128, :])
                nc.sync.dma_start(wv_sb[:, i, :], wv[i * 128 : (i + 1) * 128, :])
                nc.sync.dma_start(xt[:, i, :], xT[i * 128 : (i + 1) * 128, :])
                nc.sync.dma_start(wq_sb[:, i, :], wq[i * 128 : (i + 1) * 128, :])
            for h in range(M):
                nc.sync.dma_start(wo_sb[:, h, :], wo[h * 128 : (h + 1) * 128, :])

            qt = const.tile([128, M, T], f16)
            kt = const.tile([128, T], f16)
            vt_sb = const.tile([128, T], f16)
            v_sb = const.tile([128, NSB, 132], f16)

            # ---- projections ----
            for tc_i in range(NTCH):
                acc = psum.tile([128, TCH], f32, tag="acc", bufs=3, name="acc")
                for ks in range(NKS):
                    nc.tensor.matmul(
                        acc[:],
                        wk_sb[:, ks, :],
                        xt[:, ks, tc_i * TCH : (tc_i + 1) * TCH],
                        start=(ks == 0),
                        stop=(ks == NKS - 1),
                    )
                nc.vector.tensor_scalar_add(
                    kt[:, tc_i * TCH : (tc_i + 1) * TCH], acc[:], bks[:, 0:1]
                )
            for tc_i in range(NTCH):
                acc = psum.tile([128, TCH], f32, tag="acc", bufs=3, name="acc")
                for ks in range(NKS):
                    nc.tensor.matmul(
                        acc[:],
                        wv_sb[:, ks, :],
                        xt[:, ks, tc_i * TCH : (tc_i + 1) * TCH],
                        start=(ks == 0),
                        stop=(ks == NKS - 1),
                    )
                nc.vector.tensor_scalar_add(
                    vt_sb[:, tc_i * TCH : (tc_i + 1) * TCH], acc[:], bvs[:, 0:1]
                )
            # V natural [s, hd] + ones column for the denominator
            for s in range(NSB):
                tp = psum.tile([128, 128], f16, tag="tp", bufs=1, name="tp")
                nc.tensor.transpose(tp[:], vt_sb[:, s * 128 : (s + 1) * 128], ident[:])
                nc.vector.tensor_copy(v_sb[:, s, 0:128], tp[:])
            nc.vector.memset(v_sb[:, :, 128:129], 1.0)
            for h in range(M):
                for tc_i in range(NTCH):
                    acc = psum.tile([128, TCH], f32, tag="acc", bufs=3, name="acc")
                    for ks in range(NKS):
                        nc.tensor.matmul(
                            acc[:],
                            wq_sb[:, ks, h * 128 : (h + 1) * 128],
                            xt[:, ks, tc_i * TCH : (tc_i + 1) * TCH],
                            start=(ks == 0),
                            stop=(ks == NKS - 1),
                        )
                    nc.vector.tensor_scalar(
                        qt[:, h, tc_i * TCH : (tc_i + 1) * TCH],
                        acc[:],
                        SCALE,
                        bqs[:, h : h + 1],
                        op0=mybir.AluOpType.mult,
                        op1=mybir.AluOpType.add,
                    )

            # ---- attention + o_proj, streamed per t-chunk ----
            # per-tc fp16 partials, ReduceScatter pipelined against next chunk
            partials = [
                dram.tile([D, TCH], f16, tag=f"ptl{i}", name=f"ptl{i}")
                for i in range(NTCH)
            ]
            rss = [
                dram.tile([TCH, TCH], f16, tag=f"rs{i}", name=f"rs{i}")
                for i in range(NTCH)
            ]
            for tc_i in range(NTCH):
                at = work.tile([128, M, TCH], f16, tag="at", bufs=2, name="at")
                for h in range(M):
                    # one PSUM bank per accumulator: a matmul start zeroes the
                    # whole 2KB bank, so groups can never share a bank
                    opks = [
                        psum.tile([128, 129], f32, tag="opk", bufs=4, name=f"opk{i}")
                        for i in range(4)
                    ]

                    sps_l = [None] * NSB
                    p_l = [None] * NSB

                    def emit_score(s):
                        sps = psum.tile([128, TCH], f32, tag="acc", bufs=3, name="sps")
                        nc.tensor.matmul(
                            sps[:],
                            kt[:, s * 128 : (s + 1) * 128],
                            qt[:, h, tc_i * TCH : (tc_i + 1) * TCH],
                            start=True,
                            stop=True,
                        )
                        sps_l[s] = sps

                    def emit_exp(s):
                        p_sb = work.tile([128, TCH], f16, tag="p", bufs=4, name="p_sb")
                        nc.scalar.activation(p_sb[:], sps_l[s][:], Exp)
                        p_l[s] = p_sb
                        sps_l[s] = None

                    def emit_av(s):
                        p_sb = p_l[s]
                        for tb in range(4):
                            nc.tensor.matmul(
                                opks[tb][:, 0:129],
                                p_sb[:, tb * 128 : (tb + 1) * 128],
                                v_sb[:, s, 0:129],
                                start=(s == 0),
                                stop=(s == NSB - 1),
                            )
                        p_l[s] = None

                    # software pipeline: scores run 2 s-blocks ahead of A@V
                    DEPTH = 2
                    for s in range(NSB + DEPTH):
                        if s < NSB:
                            emit_score(s)
                            emit_exp(s)
                        if s >= DEPTH:
                            emit_av(s - DEPTH)

                    for tb in range(4):
                        opk = opks[tb]
                        off = 0
                        rcp = work.tile([128, 1], f32, tag="rcp", bufs=2, name="rcp")
                        nc.vector.reciprocal(rcp[:], opk[:, off + 128 : off + 129])
                        o_sb = work.tile([128, 128], f16, tag="osb", bufs=2, name="osb")
                        nc.vector.tensor_scalar_mul(
                            o_sb[:], opk[:, off : off + 128], rcp[:]
                        )
                        tp = psum.tile([128, 128], f16, tag="tp", bufs=1, name="tp")
                        nc.tensor.transpose(tp[:], o_sb[:], ident[:])
                        nc.vector.tensor_copy(
                            at[:, h, tb * 128 : (tb + 1) * 128], tp[:]
                        )
                # o_proj partial (transposed): partial^T[c, t] for this t-chunk
                partial = partials[tc_i]
                for cb in range(D // 128):
                    pp = psum.tile([128, TCH], f32, tag="acc", bufs=3, name="pp")
                    for h in range(M):
                        nc.tensor.matmul(
                            pp[:],
                            wo_sb[:, h, cb * 128 : (cb + 1) * 128],
                            at[:, h, :],
                            start=(h == 0),
                            stop=(h == M - 1),
                        )
                    po_sb = work.tile([128, TCH], f16, tag="po", bufs=3, name="po_sb")
                    nc.vector.tensor_scalar_add(po_sb[:], pp[:], bo4[:, cb : cb + 1])
                    nc.sync.dma_start(
                        partial[cb * 128 : (cb + 1) * 128, :],
                        po_sb[:],
                    )
                # pipelined ReduceScatter of this chunk (fp16). The only
                # consumer of the RS output is a DRAM->DRAM DMA on the idle
                # gpsimd queue, emitted one chunk late so no RS trigger ever
                # waits behind a copy; the hot engine queues never block on
                # the collective.
                nc.gpsimd.collective_compute(
                    "ReduceScatter",
                    mybir.AluOpType.add,
                    replica_groups=groups,
                    ins=[partials[tc_i][:]],
                    outs=[rss[tc_i][:]],
                )
                if tc_i > 0:
                    nc.gpsimd.dma_start(outT[tc_i - 1], rss[tc_i - 1][:])
            nc.gpsimd.dma_start(outT[NTCH - 1], rss[NTCH - 1][:])

    nc.compile()
    return nc


def _get_nc():
    if "nc" not in _COMPILED:
        _COMPILED["nc"] = _build()
    return _COMPILED["nc"]


def kernel(x, Wq, bq, Wk, bk, Wv, bv, Wo, bo):
    from concourse.bass_utils import run_bass_kernel_spmd

    x = np.asarray(x, np.float32)
    Wq = np.asarray(Wq, np.float32)
    Wk = np.asarray(Wk, np.float32)
    Wv = np.asarray(Wv, np.float32)
    Wo = np.asarray(Wo, np.float32)
    bq = np.asarray(bq, np.float32)
    bk = np.asarray(bk, np.float32)
    bv = np.asarray(bv, np.float32)
    bo = np.asarray(bo, np.float32)

    nc = _get_nc()

    in_maps = []
    for c in range(N_CORES):
        b, g = c // 4, c % 4
        in_maps.append(
            {
                "xT": np.ascontiguousarray(x[b].T).astype(np.float16),
                "wq": np.ascontiguousarray(
                    Wq[:, g * GQ : (g + 1) * GQ]
                ).astype(np.float16),
                "wk": np.ascontiguousarray(
                    Wk[:, g * HD : (g + 1) * HD]
                ).astype(np.float16),
                "wv": np.ascontiguousarray(
                    Wv[:, g * HD : (g + 1) * HD]
                ).astype(np.float16),
                "wo": np.ascontiguousarray(
                    Wo[g * GQ : (g + 1) * GQ, :]
                ).astype(np.float16),
                "bqs": np.ascontiguousarray(
                    (bq[g * GQ : (g + 1) * GQ] * SCALE).reshape(M, 128).T
                ),
                "bks": np.ascontiguousarray(
                    bk[g * HD : (g + 1) * HD].reshape(1, 128).T
                ),
                "bvs": np.ascontiguousarray(
                    bv[g * HD : (g + 1) * HD].reshape(1, 128).T
                ),
                "bo4": np.ascontiguousarray((bo / 4.0).reshape(D // 128, 128).T),
            }
        )

    res = run_bass_kernel_spmd(nc, in_maps, list(range(N_CORES)))
    _COMPILED["last_res"] = res

    out = np.empty((B, T, D), np.float32)
    for b in range(B):
        for j in range(4):
            r = res.results[4 * b + j]["outT"]  # [4, 512, 512] fp16
            for tc_i in range(NTCH):
                out[b, tc_i * TCH : (tc_i + 1) * TCH, j * TCH : (j + 1) * TCH] = (
                    r[tc_i].T
                )
    return out
